# revision 26
# baseline (speedup 1.0000x reference)
"""CTLSTM (Neural Hawkes continuous-time LSTM) Trainium2 kernel, v2.

Data-parallel over batch across 8 NeuronCores (8 batch rows per core).
Per core the T=200 recurrence is serial; per step the h@Wh matmul streams
all of Wh through the PE (LDWEIGHTS-bound, ~6us/step) while the gate math
runs on DVE/ACT. Split-half pipeline: per-half elementwise work (~2.5us)
hides under the other half's matmuls (~3.1us), so steady state is PE-bound.

v2 changes vs v1 (2.53ms): single 4-k PSUM accumulation group per m-subtile
(drops one DVE add), egx = exp(x@Wx+b) precomputed so `u = exp(P)*egx`
replaces add+exp, and four fused custom-DVE ops (BITWISE_NOT-seeded
1-Newton reciprocal) collapse sigmoid/tanh products into single ops:
  CTL_RECIP1P    out = 1/(1+x)            (sigma gates, o)
  CTL_SIGMUL     out = y/(1+x)            (i*z, ib*z, f*c_d, fb*cbar)
  CTL_NTANH      out = (x-1)/(1+x)        (-tanh from u=e^{-2g})
1-NR reciprocal max rel err 0.17%; end-to-end rel err ~2.5e-3 (host-validated).

Numerics tricks carried over from v1: bf16 weights/h/egx with fp32 psum;
single ACT LUT table (natural_log_exp: Exp/Ln) for the whole program;
weight columns pre-scaled so ONE exp covers all 7 gates (sigma * -1,
z * -2, d * +1); Wh negated because on-chip h is -h.
"""

import numpy as np
import ml_dtypes

import concourse.bass as bass
import concourse.bacc as bacc
import concourse.mybir as mybir
import concourse.tile as tile
from concourse.bass_utils import run_bass_kernel_spmd

BF16 = ml_dtypes.bfloat16

B, T, D, H = 64, 200, 256, 512
NCORES = 8
BL = B // NCORES          # 8 batch rows per core
G7 = 7 * H                # 3584 gate columns
NM = G7 // 128            # 28 M-tiles
KH = H // 128             # 4 K-tiles for Wh
KD = D // 128             # 2 K-tiles for Wx
NTB = T * BL              # 1600 (t, b) pairs
RING = 16                 # output ring slots
DMA_EVERY = 8

# new gate order (i, ib, f, fb, o, z, d) -> original split order
# (gi, gf, gz, go, gib, gfb, gd)
GATE_PERM = [0, 4, 1, 5, 3, 2, 6]
COL_SCALE = [-1.0, -1.0, -1.0, -1.0, -1.0, -2.0, 1.0]

F32 = mybir.dt.float32
BF = mybir.dt.bfloat16
AF = mybir.ActivationFunctionType
OP = mybir.AluOpType

# 1-NR BITWISE_NOT-seed reciprocal constants (minimax over the seed interval)
RC0, RC1 = -0.23549792, 2.0017324

_PROGRAM_CACHE = {}
_CTL_OPS = {}

# Engine assignment for offloadable elementwise ops: "dve" (vector) or
# "pool" (gpsimd). DVE is the bottleneck engine; Pool is otherwise idle.
OFFLOAD = {
    "t01": "pool",   # uP0*uP1          [128,112]
    "u": "dve",      # t01*egx (bf16)   [128,112]
    "e_in": "pool",  # ndt*delta        [128,16]
    "cmb": "pool",   # c - cbar         [128,16]
    "cmbE": "pool",  # cmb*E            [128,16]
    "c_d": "pool",   # cmbE + cbar      [128,16]
    "h": "dve",      # nth*o (bf16 out) [128,16]
}


def _register_ctl_ops():
    """Register the fused custom-DVE ops (idempotent per process)."""
    if _CTL_OPS:
        return _CTL_OPS
    import concourse.dve_ops as dvo
    from concourse.dve_spec import (
        Spec, Src0, Src1, Bin, AluOp, One, C0, C1, lower, _has_src1,
    )
    from concourse.dve_uop import DveOpSpec

    def notf(x):
        return (~np.asarray(x, np.float32).view(np.int32)).view(np.float32)

    def ref_recip1p(in0, in1, s0, s1, imm2):
        v = (np.asarray(in0, np.float32) + np.float32(1.0)).astype(np.float32)
        y0 = (notf(v) * np.float32(s0)).astype(np.float32)
        return (y0 * (np.float32(s1) - v * y0)).astype(np.float32)

    def ref_sigmul(in0, in1, s0, s1, imm2):
        return (np.asarray(in1, np.float32) * ref_recip1p(in0, None, s0, s1, imm2)).astype(np.float32)

    def ref_ntanh(in0, in1, s0, s1, imm2):
        return ((np.asarray(in0, np.float32) - np.float32(1.0))
                * ref_recip1p(in0, None, s0, s1, imm2)).astype(np.float32)

    def mk(name, body_fn, reference):
        if name in dvo._SUB_OPCODE_FOR_NAME:
            for op in dvo.OPS:
                if op.name == name:
                    return op
        v = Src0 + One
        nx = Bin(AluOp.BITWISE_NOT, v, v)
        y0 = nx * C0
        y1 = y0 * (C1 - v * y0)
        spec = Spec(body=body_fn(y1), reference=reference)
        op = dvo.DveOp(name=name, spec=spec, subdim=False, uops_sha={})
        dvo.OPS.append(op)
        row = dvo._CUSTOM_DVE_ROW_BASE + len(dvo.OPS) - 1
        dvo._SUB_OPCODE_FOR_NAME[name] = row
        dvo.CUSTOM_DVE_SPECS[name] = spec
        for ver in ("v3", "v4"):
            uops = lower(spec, ver=ver)
            op.uops_sha[ver] = DveOpSpec(
                name=name, opcode=row, uops=uops, rd1_en=_has_src1(spec)
            ).sha(ver)
        return op

    _CTL_OPS["RECIP1P"] = mk("CTL_RECIP1P", lambda y1: y1, ref_recip1p)
    _CTL_OPS["SIGMUL"] = mk("CTL_SIGMUL", lambda y1: y1 * Src1, ref_sigmul)
    _CTL_OPS["NTANH"] = mk("CTL_NTANH", lambda y1: (Src0 - One) * y1, ref_ntanh)
    return _CTL_OPS


class _OneTableBacc(bacc.Bacc):
    """Pin every activation to the natural_log_exp_and_others LUT table
    so the program needs exactly one ACT table load."""

    def insert_act_table_loads(self):
        from concourse.hw_specs import get_activation_tables

        has_activation = any(
            isinstance(i, mybir.InstActivation)
            for b in self.main_func.blocks
            for i in b.instructions
        )
        if not has_activation:
            return
        keep = "natural_log_exp_and_others"
        tables = [
            (n, (s if n == keep else set()))
            for n, s in get_activation_tables(self.m.arch).items()
        ]
        bacc._bass_rust.insert_act_table_loads(self, tables)


def _build_program(repeat=1, probe=False):
    ops = _register_ctl_ops()
    RECIP1P, SIGMUL, NTANH = ops["RECIP1P"], ops["SIGMUL"], ops["NTANH"]

    nc = _OneTableBacc("TRN2", target_bir_lowering=False, debug=False)

    if probe:
        # timing-probe build: tiny per-call I/O (internal DRAM inputs,
        # garbage contents — engine timing is data-independent)
        whs_d = nc.dram_tensor("whs", [128, KH * G7], BF).ap()
        wxs_d = nc.dram_tensor("wxs", [128, KD * G7], BF).ap()
        xts_d = nc.dram_tensor("xts", [128, KD * NTB], BF).ap()
        ndt_d = nc.dram_tensor("negdt", [128, T * 2 * BL], F32).ap()
        bcol_d = nc.dram_tensor("bcol", [128, NM], F32).ap()
        nc.dram_tensor("dummy_in", [128, 8], F32, kind="ExternalInput")
        out_d = nc.dram_tensor("outs", [128, T * 128], F32).ap()
        dum_o = nc.dram_tensor("dummy_out", [128, 8], F32, kind="ExternalOutput").ap()
    else:
        whs_d = nc.dram_tensor("whs", [128, KH * G7], BF, kind="ExternalInput").ap()
        wxs_d = nc.dram_tensor("wxs", [128, KD * G7], BF, kind="ExternalInput").ap()
        xts_d = nc.dram_tensor("xts", [128, KD * NTB], BF, kind="ExternalInput").ap()
        ndt_d = nc.dram_tensor("negdt", [128, T * 2 * BL], F32, kind="ExternalInput").ap()
        bcol_d = nc.dram_tensor("bcol", [128, NM], F32, kind="ExternalInput").ap()
        out_d = nc.dram_tensor("outs", [128, T * 128], F32, kind="ExternalOutput").ap()
    out_r = out_d.rearrange("p (t s) -> p t s", s=128)

    with tile.TileContext(nc) as tc:
        import contextlib

        ctx = contextlib.ExitStack()
        with ctx:
            const = ctx.enter_context(tc.tile_pool(name="const", bufs=1))
            whs = const.tile([128, KH * G7], BF, tag="whs")
            wxs = const.tile([128, KD * G7], BF, tag="wxs")
            xts = const.tile([128, KD * NTB], BF, tag="xts")
            ndt = const.tile([128, T * 2 * BL], F32, tag="ndt")
            bcol = const.tile([128, NM], F32, tag="bcol")
            egx = const.tile([128, T * 224], BF, tag="egx")
            ring64 = const.tile([128, RING * 64], F32, tag="ring64")
            ringd = const.tile([128, RING * 32], F32, tag="ringd")
            ringo = const.tile([128, RING * 32], F32, tag="ringo")

            # phase-1 inputs first so phase 1 isn't blocked behind whs/ndt
            nc.sync.dma_start(wxs[:], wxs_d)
            nc.sync.dma_start(xts[:], xts_d)
            nc.sync.dma_start(bcol[:], bcol_d)
            nc.sync.dma_start(whs[:], whs_d)
            nc.sync.dma_start(ndt[:], ndt_d)

            egx_r = egx.rearrange("p (t g) -> p t g", g=224)
            ndt_r = ndt.rearrange("p (t x) -> p t x", x=2 * BL)
            r64 = ring64.rearrange("p (s st x) -> p s st x", st=2, x=32)
            r64f = ring64.rearrange("p (s x) -> p s x", x=64)
            rd = ringd.rearrange("p (s x) -> p s x", x=32)
            ro = ringo.rearrange("p (s x) -> p s x", x=32)

            # zero-init states: step 0 reads slot RING-1
            nc.vector.memset(r64f[:, RING - 1, :], 0.0)
            nc.vector.memset(rd[:, RING - 1, :], 0.0)
            nc.vector.memset(ro[:, RING - 1, :], 0.0)

            # ---- phase 1: egx[t] = exp(x_t @ Wx' + b')^T for all t, bf16 ----
            nchunks = [(0, 512), (512, 512), (1024, 512), (1536, 64)]
            with tc.tile_pool(name="gxps", bufs=2, space="PSUM") as gxps:
                for m in range(NM):
                    g, hc = m // 4, m % 4
                    X, hcr = hc // 2, hc % 2
                    j = g * 2 + hcr
                    col = X * 112 + j * 8
                    for (n0, nsz) in nchunks:
                        ps = gxps.tile([128, 512], F32, tag="gxp")
                        for k in range(KD):
                            nc.tensor.matmul(
                                ps[:, :nsz],
                                wxs[:, k * G7 + m * 128 : k * G7 + (m + 1) * 128],
                                xts[:, k * NTB + n0 : k * NTB + n0 + nsz],
                                start=(k == 0),
                                stop=(k == KD - 1),
                            )
                        t0, nt = n0 // BL, nsz // BL
                        src = ps[:, :nsz].rearrange("p (t b) -> p t b", b=BL)
                        dst = egx_r[:, t0 : t0 + nt, col : col + 8]
                        nc.scalar.activation(dst, src, AF.Exp, bias=bcol[:, m : m + 1])

            # ---- phase 2: the recurrence ----
            sp = ctx.enter_context(tc.tile_pool(name="sp", bufs=2))
            hp = ctx.enter_context(tc.tile_pool(name="hp", bufs=3))
            cdp = ctx.enter_context(tc.tile_pool(name="cdp", bufs=3))
            psp = ctx.enter_context(tc.tile_pool(name="psp", bufs=2, space="PSUM"))

            def pe_half(ps, h, X, kp):
                # 14 m-subtiles; self-contained 2-matmul group per (j, kpair).
                # kp=0 groups only read the A-half of h (decay-A), so the PE
                # can start next step's kp0 pass before decay-B lands.
                for j in range(14):
                    g, hcr = j // 2, j % 2
                    m = g * 4 + 2 * X + hcr
                    for k in (2 * kp, 2 * kp + 1):
                        nc.tensor.matmul(
                            ps[:, j * BL : (j + 1) * BL],
                            whs[:, k * G7 + m * 128 : k * G7 + (m + 1) * 128],
                            h[:, k * BL : (k + 1) * BL],
                            start=(k == 2 * kp),
                            stop=(k == 2 * kp + 1),
                        )

            def eng(which):
                return nc.gpsimd if OFFLOAD[which] == "pool" else nc.vector

            def prefetch_E(tn, X, dslot):
                # E(tn) = exp(-dt_tn * delta) with delta read from ring slot
                # dslot; emitted early so the ACT round-trip hides under the
                # remaining gate math.
                e_in = sp.tile([128, 16], F32, tag=f"e_in{X}")
                eng("e_in").tensor_mul(e_in[:], ndt_r[:, tn % T, :], rd[:, dslot, slice(X * 16, X * 16 + 16)])
                E = sp.tile([128, 16], F32, tag=f"E{X}")
                nc.scalar.activation(E[:], e_in[:], AF.Exp)
                return E

            def gates(t, X, ps0, ps1, cd_prev, with_E=True):
                # u layout (16 cols per group): i ib f fb o z d
                slot, prev = t % RING, (t - 1) % RING
                c16 = slice(X * 16, X * 16 + 16)
                uP0 = sp.tile([128, 112], F32, tag=f"uP0{X}")
                nc.scalar.activation(uP0[:], ps0[:], AF.Exp)
                uP1 = sp.tile([128, 112], F32, tag=f"uP1{X}")
                nc.scalar.activation(uP1[:], ps1[:], AF.Exp)
                t01 = sp.tile([128, 112], F32, tag=f"t01{X}")
                eng("t01").tensor_mul(t01[:], uP0[:], uP1[:])
                u = sp.tile([128, 112], F32, tag=f"u{X}")
                eng("u").tensor_mul(u[:], t01[:], egx_r[:, t, X * 112 : X * 112 + 112])
                nc.scalar.activation(rd[:, slot, c16], u[:, 96:112], AF.Ln, bias=1.0)
                zz = sp.tile([128, 16], F32, tag=f"zz{X}")  # = -z
                nc.vector._custom_dve(NTANH, out=zz[:], in0=u[:, 80:96], s0=RC0, s1=RC1)
                TI = sp.tile([128, 32], F32, tag=f"TI{X}")  # [i*zz | ib*zz]
                nc.vector._custom_dve(SIGMUL, out=TI[:, 0:16], in0=u[:, 0:16], in1=zz[:], s0=RC0, s1=RC1)
                nc.vector._custom_dve(SIGMUL, out=TI[:, 16:32], in0=u[:, 16:32], in1=zz[:], s0=RC0, s1=RC1)
                # cd_prev = [c_d | cbar(t-1)] assembled by decay, so one
                # 32-wide fused op covers both P2 halves
                P2 = sp.tile([128, 32], F32, tag=f"P2{X}")  # [f*c_d | fb*cbar]
                nc.vector._custom_dve(SIGMUL, out=P2[:], in0=u[:, 32:64], in1=cd_prev[:], s0=RC0, s1=RC1)
                # c_new = f*c_d + i*z ; cbar_new = fb*cbar + ib*z  (z = -zz)
                P2r = P2.rearrange("p (st x) -> p st x", st=2)
                TIr = TI.rearrange("p (st x) -> p st x", st=2)
                nc.vector.tensor_sub(r64[:, slot, 0:2, c16], P2r[:], TIr[:])
                nc.vector._custom_dve(RECIP1P, out=ro[:, slot, c16], in0=u[:, 64:80], s0=RC0, s1=RC1)
                # E for the next step's decay, emitted last so it doesn't
                # stall the in-order DVE queue (delta is long since ready)
                return prefetch_E(t + 1, X, slot) if with_E else None

            def decay(tn, X, h, E):
                # produces h(tn) half X and c_d(tn); reads state at tn-1
                prev = (tn - 1) % RING
                c16 = slice(X * 16, X * 16 + 16)
                cmb = sp.tile([128, 16], F32, tag=f"cmb{X}")
                eng("cmb").tensor_sub(cmb[:], r64[:, prev, 0, c16], r64[:, prev, 1, c16])
                cmbE = sp.tile([128, 16], F32, tag=f"cmbE{X}")
                eng("cmbE").tensor_mul(cmbE[:], cmb[:], E[:])
                # CD2 = [c_d | cbar]: feeds both the h path and next step's
                # 32-wide P2; the cbar copy is off the critical path
                CD2 = cdp.tile([128, 32], F32, tag=f"cd{X}")
                eng("c_d").tensor_add(CD2[:, 0:16], cmbE[:], r64[:, prev, 1, c16])
                nc.gpsimd.tensor_copy(CD2[:, 16:32], r64[:, prev, 1, c16])
                u_c = sp.tile([128, 16], F32, tag=f"u_c{X}")
                nc.scalar.activation(u_c[:], CD2[:, 0:16], AF.Exp, scale=-2.0)
                nth = sp.tile([128, 16], F32, tag=f"nth{X}")  # = -tanh(c_d)
                nc.vector._custom_dve(NTANH, out=nth[:], in0=u_c[:], s0=RC0, s1=RC1)
                # h' = -o*tanh(c_d)  (sign absorbed by negated Wh)
                eng("h").tensor_mul(h[:, c16], nth[:], ro[:, prev, c16])
                return CD2

            h_prev = hp.tile([128, 4 * BL], BF, tag="h")
            EA = prefetch_E(0, 0, RING - 1)
            cdA = decay(0, 0, h_prev, EA)
            EB = prefetch_E(0, 1, RING - 1)
            cdB = decay(0, 1, h_prev, EB)
            nsteps = repeat * T
            for it in range(nsteps):
                t = it % T
                last = it == nsteps - 1
                psA0 = psp.tile([128, 112], F32, tag="psA0")
                psA1 = psp.tile([128, 112], F32, tag="psA1")
                psB0 = psp.tile([128, 112], F32, tag="psB0")
                psB1 = psp.tile([128, 112], F32, tag="psB1")
                # A pieces first: psA completes at ~half the PE time, so the
                # A-half EW chain (the critical cycle) starts as early as
                # possible; B's kp0 still only needs the A-half of h.
                pe_half(psA0, h_prev, 0, 0)
                pe_half(psA1, h_prev, 0, 1)
                pe_half(psB0, h_prev, 1, 0)
                pe_half(psB1, h_prev, 1, 1)

                tn = (t + 1) % T
                # decay writes h(t+1)/c_d(t+1); skipped on the final step
                # (nothing would read them, and the verifier rejects dead tiles)
                if not last:
                    h_next = hp.tile([128, 4 * BL], BF, tag="h")
                EA = gates(t, 0, psA0, psA1, cdA, with_E=not last)
                if not last:
                    cdA = decay(tn, 0, h_next, EA)
                EB = gates(t, 1, psB0, psB1, cdB, with_E=not last)
                if not last:
                    cdB = decay(tn, 1, h_next, EB)
                    h_prev = h_next

                slot = t % RING
                if t % DMA_EVERY == DMA_EVERY - 1:
                    lo = slot - (DMA_EVERY - 1)
                    t0 = t - (DMA_EVERY - 1)
                    nc.sync.dma_start(
                        out_r[:, t0 : t + 1, 0:64], r64f[:, lo : slot + 1, :]
                    )
                    nc.sync.dma_start(
                        out_r[:, t0 : t + 1, 64:96], rd[:, lo : slot + 1, :]
                    )
                    nc.sync.dma_start(
                        out_r[:, t0 : t + 1, 96:128], ro[:, lo : slot + 1, :]
                    )

            if probe:
                nc.sync.dma_start(dum_o, r64f[:, (T - 1) % RING, 0:8])

    nc.compile()
    return nc


def _get_program():
    if "nc" not in _PROGRAM_CACHE:
        _PROGRAM_CACHE["nc"] = _build_program()
    return _PROGRAM_CACHE["nc"]


def _prep_shared(Wx, Wh, b):
    perm = np.concatenate([g * H + np.arange(H) for g in GATE_PERM])
    scale = np.repeat(np.array(COL_SCALE, np.float32), H)
    WxP = (Wx[:, perm] * scale).astype(np.float32)
    WhP = (-(Wh[:, perm] * scale)).astype(np.float32)
    bP = (b[perm] * scale).astype(np.float32)
    whs = np.ascontiguousarray(
        WhP.reshape(KH, 128, G7).transpose(1, 0, 2).reshape(128, KH * G7)
    ).astype(BF16)
    wxs = np.ascontiguousarray(
        WxP.reshape(KD, 128, G7).transpose(1, 0, 2).reshape(128, KD * G7)
    ).astype(BF16)
    bcol = np.ascontiguousarray(bP.reshape(NM, 128).T).astype(np.float32)
    return whs, wxs, bcol


def make_in_maps(input_, duration, Wx, Wh, b):
    X = np.asarray(input_, np.float32)
    dur = np.asarray(duration, np.float32)
    whs, wxs, bcol = _prep_shared(
        np.asarray(Wx, np.float32), np.asarray(Wh, np.float32), np.asarray(b, np.float32)
    )
    in_maps = []
    for ci in range(NCORES):
        Xc = X[ci * BL : (ci + 1) * BL]              # (BL, T, D)
        xts = np.ascontiguousarray(
            Xc.transpose(2, 1, 0).reshape(KD, 128, NTB).transpose(1, 0, 2).reshape(128, KD * NTB)
        ).astype(BF16)
        ndc = -dur[ci * BL : (ci + 1) * BL].T        # (T, BL)
        negdt = np.ascontiguousarray(
            np.broadcast_to(ndc[None, :, None, :], (128, T, 2, BL)).reshape(128, T * 2 * BL)
        ).astype(np.float32)
        in_maps.append(
            {"whs": whs, "wxs": wxs, "xts": xts, "negdt": negdt, "bcol": bcol}
        )
    return in_maps


def assemble_output(results):
    full = np.empty((4, B, T, H), np.float32)
    for ci in range(NCORES):
        arr = np.asarray(results[ci]["outs"]).reshape(128, T, 4, 4, BL)
        # arr[p, t, state, hc, b] -> full[state, b, t, hc*128 + p]
        full[:, ci * BL : (ci + 1) * BL] = (
            arr.transpose(2, 4, 1, 3, 0).reshape(4, BL, T, H)
        )
    return full


def kernel(**inputs):
    nc = _get_program()
    in_maps = make_in_maps(
        inputs["input_"], inputs["duration"], inputs["Wx"], inputs["Wh"], inputs["b"]
    )
    res = run_bass_kernel_spmd(nc, in_maps, list(range(NCORES)))
    return assemble_output(res.results)


# revision 27
# speedup vs baseline: 1.4132x; 1.4132x over previous
"""CTLSTM (Neural Hawkes continuous-time LSTM) Trainium2 kernel, v2.

Data-parallel over batch across 8 NeuronCores (8 batch rows per core).
Per core the T=200 recurrence is serial; per step the h@Wh matmul streams
all of Wh through the PE (LDWEIGHTS-bound, ~6us/step) while the gate math
runs on DVE/ACT. Split-half pipeline: per-half elementwise work (~2.5us)
hides under the other half's matmuls (~3.1us), so steady state is PE-bound.

v2 changes vs v1 (2.53ms): single 4-k PSUM accumulation group per m-subtile
(drops one DVE add), egx = exp(x@Wx+b) precomputed so `u = exp(P)*egx`
replaces add+exp, and four fused custom-DVE ops (BITWISE_NOT-seeded
1-Newton reciprocal) collapse sigmoid/tanh products into single ops:
  CTL_RECIP1P    out = 1/(1+x)            (sigma gates, o)
  CTL_SIGMUL     out = y/(1+x)            (i*z, ib*z, f*c_d, fb*cbar)
  CTL_NTANH      out = (x-1)/(1+x)        (-tanh from u=e^{-2g})
1-NR reciprocal max rel err 0.17%; end-to-end rel err ~2.5e-3 (host-validated).

Numerics tricks carried over from v1: bf16 weights/h/egx with fp32 psum;
single ACT LUT table (natural_log_exp: Exp/Ln) for the whole program;
weight columns pre-scaled so ONE exp covers all 7 gates (sigma * -1,
z * -2, d * +1); Wh negated because on-chip h is -h.
"""

import numpy as np
import ml_dtypes

import concourse.bass as bass
import concourse.bacc as bacc
import concourse.mybir as mybir
import concourse.tile as tile
from concourse.bass_utils import run_bass_kernel_spmd

BF16 = ml_dtypes.bfloat16

B, T, D, H = 64, 200, 256, 512
NCORES = 8
BL = B // NCORES          # 8 batch rows per core
G7 = 7 * H                # 3584 gate columns
NM = G7 // 128            # 28 M-tiles
KH = H // 128             # 4 K-tiles for Wh
KD = D // 128             # 2 K-tiles for Wx
NTB = T * BL              # 1600 (t, b) pairs
RING = 16                 # output ring slots
DMA_EVERY = 8

# new gate order (i, ib, f, fb, o, z, d) -> original split order
# (gi, gf, gz, go, gib, gfb, gd)
GATE_PERM = [0, 4, 1, 5, 3, 2, 6]
COL_SCALE = [-1.0, -1.0, -1.0, -1.0, -1.0, -2.0, 1.0]

F32 = mybir.dt.float32
BF = mybir.dt.bfloat16
AF = mybir.ActivationFunctionType
OP = mybir.AluOpType

# 1-NR BITWISE_NOT-seed reciprocal constants (minimax over the seed interval)
RC0, RC1 = -0.23549792, 2.0017324

_PROGRAM_CACHE = {}
_CTL_OPS = {}

# Engine assignment for offloadable elementwise ops: "dve" (vector) or
# "pool" (gpsimd). DVE is the bottleneck engine; Pool is otherwise idle.
OFFLOAD = {
    "t01": "dve",    # uP0*uP1          [128,112]
    "u": "dve",      # t01*egx (bf16)   [128,112]
    "e_in": "pool",  # ndt*delta        [128,16]  (off-chain: E is prefetched)
    "cmb": "dve",    # c - cbar         [128,16]  (on-chain: keep hop-free)
    "cmbE": "dve",   # cmb*E            [128,16]
    "c_d": "dve",    # cmbE + cbar      [128,16]
    "h": "dve",      # nth*o (bf16 out) [128,16]
}


def _register_ctl_ops():
    """Register the fused custom-DVE ops (idempotent per process)."""
    if _CTL_OPS:
        return _CTL_OPS
    import concourse.dve_ops as dvo
    from concourse.dve_spec import (
        Spec, Src0, Src1, Bin, AluOp, One, C0, C1, lower, _has_src1,
    )
    from concourse.dve_uop import DveOpSpec

    def notf(x):
        return (~np.asarray(x, np.float32).view(np.int32)).view(np.float32)

    def ref_recip1p(in0, in1, s0, s1, imm2):
        v = (np.asarray(in0, np.float32) + np.float32(1.0)).astype(np.float32)
        y0 = (notf(v) * np.float32(s0)).astype(np.float32)
        return (y0 * (np.float32(s1) - v * y0)).astype(np.float32)

    def ref_sigmul(in0, in1, s0, s1, imm2):
        return (np.asarray(in1, np.float32) * ref_recip1p(in0, None, s0, s1, imm2)).astype(np.float32)

    def ref_ntanh(in0, in1, s0, s1, imm2):
        return ((np.asarray(in0, np.float32) - np.float32(1.0))
                * ref_recip1p(in0, None, s0, s1, imm2)).astype(np.float32)

    def mk(name, body_fn, reference):
        if name in dvo._SUB_OPCODE_FOR_NAME:
            for op in dvo.OPS:
                if op.name == name:
                    return op
        v = Src0 + One
        nx = Bin(AluOp.BITWISE_NOT, v, v)
        y0 = nx * C0
        y1 = y0 * (C1 - v * y0)
        spec = Spec(body=body_fn(y1), reference=reference)
        op = dvo.DveOp(name=name, spec=spec, subdim=False, uops_sha={})
        dvo.OPS.append(op)
        row = dvo._CUSTOM_DVE_ROW_BASE + len(dvo.OPS) - 1
        dvo._SUB_OPCODE_FOR_NAME[name] = row
        dvo.CUSTOM_DVE_SPECS[name] = spec
        for ver in ("v3", "v4"):
            uops = lower(spec, ver=ver)
            op.uops_sha[ver] = DveOpSpec(
                name=name, opcode=row, uops=uops, rd1_en=_has_src1(spec)
            ).sha(ver)
        return op

    _CTL_OPS["RECIP1P"] = mk("CTL_RECIP1P", lambda y1: y1, ref_recip1p)
    _CTL_OPS["SIGMUL"] = mk("CTL_SIGMUL", lambda y1: y1 * Src1, ref_sigmul)
    _CTL_OPS["NTANH"] = mk("CTL_NTANH", lambda y1: (Src0 - One) * y1, ref_ntanh)
    return _CTL_OPS


class _OneTableBacc(bacc.Bacc):
    """Pin every activation to the natural_log_exp_and_others LUT table
    so the program needs exactly one ACT table load."""

    def insert_act_table_loads(self):
        from concourse.hw_specs import get_activation_tables

        has_activation = any(
            isinstance(i, mybir.InstActivation)
            for b in self.main_func.blocks
            for i in b.instructions
        )
        if not has_activation:
            return
        keep = "natural_log_exp_and_others"
        tables = [
            (n, (s if n == keep else set()))
            for n, s in get_activation_tables(self.m.arch).items()
        ]
        bacc._bass_rust.insert_act_table_loads(self, tables)


def _build_program(repeat=1, probe=False):
    ops = _register_ctl_ops()
    RECIP1P, SIGMUL, NTANH = ops["RECIP1P"], ops["SIGMUL"], ops["NTANH"]

    nc = _OneTableBacc("TRN2", target_bir_lowering=False, debug=False)

    if probe:
        # timing-probe build: tiny per-call I/O (internal DRAM inputs,
        # garbage contents — engine timing is data-independent)
        whs_d = nc.dram_tensor("whs", [128, KH * G7], BF).ap()
        wxs_d = nc.dram_tensor("wxs", [128, KD * G7], BF).ap()
        xts_d = nc.dram_tensor("xts", [128, KD * NTB], BF).ap()
        ndt_d = nc.dram_tensor("negdt", [128, T * 2 * BL], F32).ap()
        bcol_d = nc.dram_tensor("bcol", [128, NM], F32).ap()
        nc.dram_tensor("dummy_in", [128, 8], F32, kind="ExternalInput")
        out_d = nc.dram_tensor("outs", [128, T * 128], F32).ap()
        dum_o = nc.dram_tensor("dummy_out", [128, 8], F32, kind="ExternalOutput").ap()
    else:
        whs_d = nc.dram_tensor("whs", [128, KH * G7], BF, kind="ExternalInput").ap()
        wxs_d = nc.dram_tensor("wxs", [128, KD * G7], BF, kind="ExternalInput").ap()
        xts_d = nc.dram_tensor("xts", [128, KD * NTB], BF, kind="ExternalInput").ap()
        ndt_d = nc.dram_tensor("negdt", [128, T * 2 * BL], F32, kind="ExternalInput").ap()
        bcol_d = nc.dram_tensor("bcol", [128, NM], F32, kind="ExternalInput").ap()
        out_d = nc.dram_tensor("outs", [128, T * 128], F32, kind="ExternalOutput").ap()
    out_r = out_d.rearrange("p (t s) -> p t s", s=128)

    with tile.TileContext(nc) as tc:
        import contextlib

        ctx = contextlib.ExitStack()
        with ctx:
            const = ctx.enter_context(tc.tile_pool(name="const", bufs=1))
            whs = const.tile([128, KH * G7], BF, tag="whs")
            wxs = const.tile([128, KD * G7], BF, tag="wxs")
            xts = const.tile([128, KD * NTB], BF, tag="xts")
            ndt = const.tile([128, T * 2 * BL], F32, tag="ndt")
            bcol = const.tile([128, NM], F32, tag="bcol")
            egx = const.tile([128, T * 224], BF, tag="egx")
            ring64 = const.tile([128, RING * 64], F32, tag="ring64")
            ringd = const.tile([128, RING * 32], F32, tag="ringd")
            ringo = const.tile([128, RING * 32], F32, tag="ringo")

            # phase-1 inputs first so phase 1 isn't blocked behind whs/ndt
            nc.sync.dma_start(wxs[:], wxs_d)
            nc.sync.dma_start(xts[:], xts_d)
            nc.sync.dma_start(bcol[:], bcol_d)
            nc.sync.dma_start(whs[:], whs_d)
            nc.sync.dma_start(ndt[:], ndt_d)

            egx_r = egx.rearrange("p (t g) -> p t g", g=224)
            ndt_r = ndt.rearrange("p (t x) -> p t x", x=2 * BL)
            r64 = ring64.rearrange("p (s st x) -> p s st x", st=2, x=32)
            r64f = ring64.rearrange("p (s x) -> p s x", x=64)
            rd = ringd.rearrange("p (s x) -> p s x", x=32)
            ro = ringo.rearrange("p (s x) -> p s x", x=32)

            # zero-init states: step 0 reads slot RING-1
            nc.vector.memset(r64f[:, RING - 1, :], 0.0)
            nc.vector.memset(rd[:, RING - 1, :], 0.0)
            nc.vector.memset(ro[:, RING - 1, :], 0.0)

            # ---- phase 1: egx[t] = exp(x_t @ Wx' + b')^T for all t, bf16 ----
            nchunks = [(0, 512), (512, 512), (1024, 512), (1536, 64)]
            with tc.tile_pool(name="gxps", bufs=2, space="PSUM") as gxps:
                for m in range(NM):
                    g, hc = m // 4, m % 4
                    X, hcr = hc // 2, hc % 2
                    j = g * 2 + hcr
                    col = X * 112 + j * 8
                    for (n0, nsz) in nchunks:
                        ps = gxps.tile([128, 512], F32, tag="gxp")
                        for k in range(KD):
                            nc.tensor.matmul(
                                ps[:, :nsz],
                                wxs[:, k * G7 + m * 128 : k * G7 + (m + 1) * 128],
                                xts[:, k * NTB + n0 : k * NTB + n0 + nsz],
                                start=(k == 0),
                                stop=(k == KD - 1),
                            )
                        t0, nt = n0 // BL, nsz // BL
                        src = ps[:, :nsz].rearrange("p (t b) -> p t b", b=BL)
                        dst = egx_r[:, t0 : t0 + nt, col : col + 8]
                        nc.scalar.activation(dst, src, AF.Exp, bias=bcol[:, m : m + 1])

            # ---- phase 2: the recurrence ----
            sp = ctx.enter_context(tc.tile_pool(name="sp", bufs=2))
            hp = ctx.enter_context(tc.tile_pool(name="hp", bufs=3))
            cdp = ctx.enter_context(tc.tile_pool(name="cdp", bufs=3))
            psp = ctx.enter_context(tc.tile_pool(name="psp", bufs=2, space="PSUM"))

            def pe_half(ps, h, X, kp):
                # 14 m-subtiles; self-contained 2-matmul group per (j, kpair).
                # kp=0 groups only read the A-half of h (decay-A), so the PE
                # can start next step's kp0 pass before decay-B lands.
                for j in range(14):
                    g, hcr = j // 2, j % 2
                    m = g * 4 + 2 * X + hcr
                    for k in (2 * kp, 2 * kp + 1):
                        nc.tensor.matmul(
                            ps[:, j * BL : (j + 1) * BL],
                            whs[:, k * G7 + m * 128 : k * G7 + (m + 1) * 128],
                            h[:, k * BL : (k + 1) * BL],
                            start=(k == 2 * kp),
                            stop=(k == 2 * kp + 1),
                        )

            def eng(which):
                return nc.gpsimd if OFFLOAD[which] == "pool" else nc.vector

            def prefetch_E(tn, X, dslot):
                # E(tn) = exp(-dt_tn * delta) with delta read from ring slot
                # dslot; emitted early so the ACT round-trip hides under the
                # remaining gate math.
                e_in = sp.tile([128, 16], F32, tag=f"e_in{X}")
                eng("e_in").tensor_mul(e_in[:], ndt_r[:, tn % T, :], rd[:, dslot, slice(X * 16, X * 16 + 16)])
                E = sp.tile([128, 16], F32, tag=f"E{X}")
                nc.scalar.activation(E[:], e_in[:], AF.Exp)
                return E

            def gates(t, X, ps0, ps1, cd_prev, with_E=True):
                # u layout (16 cols per group): i ib f fb o z d
                slot, prev = t % RING, (t - 1) % RING
                c16 = slice(X * 16, X * 16 + 16)
                uP0 = sp.tile([128, 112], F32, tag=f"uP0{X}")
                nc.scalar.activation(uP0[:], ps0[:], AF.Exp)
                uP1 = sp.tile([128, 112], F32, tag=f"uP1{X}")
                nc.scalar.activation(uP1[:], ps1[:], AF.Exp)
                t01 = sp.tile([128, 112], F32, tag=f"t01{X}")
                eng("t01").tensor_mul(t01[:], uP0[:], uP1[:])
                u = sp.tile([128, 112], F32, tag=f"u{X}")
                eng("u").tensor_mul(u[:], t01[:], egx_r[:, t, X * 112 : X * 112 + 112])
                nc.scalar.activation(rd[:, slot, c16], u[:, 96:112], AF.Ln, bias=1.0)
                zz = sp.tile([128, 16], F32, tag=f"zz{X}")  # = -z
                nc.vector._custom_dve(NTANH, out=zz[:], in0=u[:, 80:96], s0=RC0, s1=RC1)
                TI = sp.tile([128, 32], F32, tag=f"TI{X}")  # [i*zz | ib*zz]
                nc.vector._custom_dve(SIGMUL, out=TI[:, 0:16], in0=u[:, 0:16], in1=zz[:], s0=RC0, s1=RC1)
                nc.vector._custom_dve(SIGMUL, out=TI[:, 16:32], in0=u[:, 16:32], in1=zz[:], s0=RC0, s1=RC1)
                # cd_prev = [c_d | cbar(t-1)] assembled by decay, so one
                # 32-wide fused op covers both P2 halves
                P2 = sp.tile([128, 32], F32, tag=f"P2{X}")  # [f*c_d | fb*cbar]
                nc.vector._custom_dve(SIGMUL, out=P2[:], in0=u[:, 32:64], in1=cd_prev[:], s0=RC0, s1=RC1)
                # c_new = f*c_d + i*z ; cbar_new = fb*cbar + ib*z  (z = -zz)
                P2r = P2.rearrange("p (st x) -> p st x", st=2)
                TIr = TI.rearrange("p (st x) -> p st x", st=2)
                nc.vector.tensor_sub(r64[:, slot, 0:2, c16], P2r[:], TIr[:])
                nc.vector._custom_dve(RECIP1P, out=ro[:, slot, c16], in0=u[:, 64:80], s0=RC0, s1=RC1)
                # E for the next step's decay, emitted last so it doesn't
                # stall the in-order DVE queue (delta is long since ready)
                return prefetch_E(t + 1, X, slot) if with_E else None

            def decay(tn, X, h, E):
                # produces h(tn) half X and c_d(tn); reads state at tn-1
                prev = (tn - 1) % RING
                c16 = slice(X * 16, X * 16 + 16)
                cmb = sp.tile([128, 16], F32, tag=f"cmb{X}")
                eng("cmb").tensor_sub(cmb[:], r64[:, prev, 0, c16], r64[:, prev, 1, c16])
                cmbE = sp.tile([128, 16], F32, tag=f"cmbE{X}")
                eng("cmbE").tensor_mul(cmbE[:], cmb[:], E[:])
                # CD2 = [c_d | cbar]: feeds both the h path and next step's
                # 32-wide P2; the cbar copy is off the critical path
                CD2 = cdp.tile([128, 32], F32, tag=f"cd{X}")
                eng("c_d").tensor_add(CD2[:, 0:16], cmbE[:], r64[:, prev, 1, c16])
                nc.gpsimd.tensor_copy(CD2[:, 16:32], r64[:, prev, 1, c16])
                u_c = sp.tile([128, 16], F32, tag=f"u_c{X}")
                nc.scalar.activation(u_c[:], CD2[:, 0:16], AF.Exp, scale=-2.0)
                nth = sp.tile([128, 16], F32, tag=f"nth{X}")  # = -tanh(c_d)
                nc.vector._custom_dve(NTANH, out=nth[:], in0=u_c[:], s0=RC0, s1=RC1)
                # h' = -o*tanh(c_d)  (sign absorbed by negated Wh)
                eng("h").tensor_mul(h[:, c16], nth[:], ro[:, prev, c16])
                return CD2

            h_prev = hp.tile([128, 4 * BL], BF, tag="h")
            EA = prefetch_E(0, 0, RING - 1)
            cdA = decay(0, 0, h_prev, EA)
            EB = prefetch_E(0, 1, RING - 1)
            cdB = decay(0, 1, h_prev, EB)
            nsteps = repeat * T
            for it in range(nsteps):
                t = it % T
                last = it == nsteps - 1
                psA0 = psp.tile([128, 112], F32, tag="psA0")
                psA1 = psp.tile([128, 112], F32, tag="psA1")
                psB0 = psp.tile([128, 112], F32, tag="psB0")
                psB1 = psp.tile([128, 112], F32, tag="psB1")
                # A pieces first: psA completes at ~half the PE time, so the
                # A-half EW chain (the critical cycle) starts as early as
                # possible; B's kp0 still only needs the A-half of h.
                pe_half(psA0, h_prev, 0, 0)
                pe_half(psA1, h_prev, 0, 1)
                pe_half(psB0, h_prev, 1, 0)
                pe_half(psB1, h_prev, 1, 1)

                tn = (t + 1) % T
                # decay writes h(t+1)/c_d(t+1); skipped on the final step
                # (nothing would read them, and the verifier rejects dead tiles)
                if not last:
                    h_next = hp.tile([128, 4 * BL], BF, tag="h")
                EA = gates(t, 0, psA0, psA1, cdA, with_E=not last)
                if not last:
                    cdA = decay(tn, 0, h_next, EA)
                EB = gates(t, 1, psB0, psB1, cdB, with_E=not last)
                if not last:
                    cdB = decay(tn, 1, h_next, EB)
                    h_prev = h_next

                slot = t % RING
                if t % DMA_EVERY == DMA_EVERY - 1:
                    lo = slot - (DMA_EVERY - 1)
                    t0 = t - (DMA_EVERY - 1)
                    nc.sync.dma_start(
                        out_r[:, t0 : t + 1, 0:64], r64f[:, lo : slot + 1, :]
                    )
                    nc.sync.dma_start(
                        out_r[:, t0 : t + 1, 64:96], rd[:, lo : slot + 1, :]
                    )
                    nc.sync.dma_start(
                        out_r[:, t0 : t + 1, 96:128], ro[:, lo : slot + 1, :]
                    )

            if probe:
                nc.sync.dma_start(dum_o, r64f[:, (T - 1) % RING, 0:8])

    nc.compile()
    return nc


def _get_program():
    if "nc" not in _PROGRAM_CACHE:
        _PROGRAM_CACHE["nc"] = _build_program()
    return _PROGRAM_CACHE["nc"]


def _prep_shared(Wx, Wh, b):
    perm = np.concatenate([g * H + np.arange(H) for g in GATE_PERM])
    scale = np.repeat(np.array(COL_SCALE, np.float32), H)
    WxP = (Wx[:, perm] * scale).astype(np.float32)
    WhP = (-(Wh[:, perm] * scale)).astype(np.float32)
    bP = (b[perm] * scale).astype(np.float32)
    whs = np.ascontiguousarray(
        WhP.reshape(KH, 128, G7).transpose(1, 0, 2).reshape(128, KH * G7)
    ).astype(BF16)
    wxs = np.ascontiguousarray(
        WxP.reshape(KD, 128, G7).transpose(1, 0, 2).reshape(128, KD * G7)
    ).astype(BF16)
    bcol = np.ascontiguousarray(bP.reshape(NM, 128).T).astype(np.float32)
    return whs, wxs, bcol


def make_in_maps(input_, duration, Wx, Wh, b):
    X = np.asarray(input_, np.float32)
    dur = np.asarray(duration, np.float32)
    whs, wxs, bcol = _prep_shared(
        np.asarray(Wx, np.float32), np.asarray(Wh, np.float32), np.asarray(b, np.float32)
    )
    in_maps = []
    for ci in range(NCORES):
        Xc = X[ci * BL : (ci + 1) * BL]              # (BL, T, D)
        xts = np.ascontiguousarray(
            Xc.transpose(2, 1, 0).reshape(KD, 128, NTB).transpose(1, 0, 2).reshape(128, KD * NTB)
        ).astype(BF16)
        ndc = -dur[ci * BL : (ci + 1) * BL].T        # (T, BL)
        negdt = np.ascontiguousarray(
            np.broadcast_to(ndc[None, :, None, :], (128, T, 2, BL)).reshape(128, T * 2 * BL)
        ).astype(np.float32)
        in_maps.append(
            {"whs": whs, "wxs": wxs, "xts": xts, "negdt": negdt, "bcol": bcol}
        )
    return in_maps


def assemble_output(results):
    full = np.empty((4, B, T, H), np.float32)
    for ci in range(NCORES):
        arr = np.asarray(results[ci]["outs"]).reshape(128, T, 4, 4, BL)
        # arr[p, t, state, hc, b] -> full[state, b, t, hc*128 + p]
        full[:, ci * BL : (ci + 1) * BL] = (
            arr.transpose(2, 4, 1, 3, 0).reshape(4, BL, T, H)
        )
    return full


def kernel(**inputs):
    nc = _get_program()
    in_maps = make_in_maps(
        inputs["input_"], inputs["duration"], inputs["Wx"], inputs["Wh"], inputs["b"]
    )
    res = run_bass_kernel_spmd(nc, in_maps, list(range(NCORES)))
    return assemble_output(res.results)
